# revision 8
# baseline (speedup 1.0000x reference)
"""Multi-head attention (B=4, S=2048, D=1024, H=16) on 8 TRN2 NeuronCores.

Sharding: core c = 2*b + g handles batch b (of 4) and head-group g (of 2,
8 heads / 512 model dims each).  Per core (all matmuls bf16, fp32 PSUM):
  - QKV projections for its batch restricted to its 512 output dims;
    qhT/khT [512, 2048] and vh [2048, 520] stay resident in SBUF
  - attention for its 8 heads in transposed-scores layout (scoresT[k, q]):
    softmax denominator via a ones-column appended to V; no max subtraction
    (scores are ~N(0, 0.08^2) after the 1/32 scale, exp cannot overflow)
  - structured to keep ScalarE (exp) saturated: per (head-pair, 512-wide
    q chunk) the two heads' scores land in one [128, 1024] PSUM tile
    (different banks, written by two concurrent 64x128 row-tiled matmuls)
    so one ACT instruction covers both heads; sc double-buffered; K/Q
    projections for later pairs and the output projection are emitted as
    "filler" chunks inside the attention kb loop to fill PE slack
  - output projection partial over its 512 model dims; partials
    ReduceScatter'd pairwise in 8 chunks so the collective overlaps compute
Host: pre-transposes inputs/weights (bf16), feeds per-core shards, and
reassembles the full [4, 2048, 1024] fp32 output from the 8 per-core
outputs (chunked-RS row interleaving: core 2b+g holds rows
256*ch + [128*g, 128*(g+1)) of batch b for ch in 0..7).
"""

import numpy as np
import ml_dtypes

import concourse.bass as bass
import concourse.mybir as mybir
import concourse.tile as tile
from concourse import bacc
from concourse.bass_utils import run_bass_kernel_spmd

N_CORES = 8
S = 2048          # sequence length
D = 1024          # d_model
DL = 512          # local model dims (8 heads x 64)
NH = 8            # local heads
DH = 64           # head dim
W = 512           # q-chunk width for attention
NQC = S // W      # 4 q chunks
SCALE = 1.0 / 32.0  # 1/sqrt(d_model)

F32 = mybir.dt.float32
BF16 = mybir.dt.bfloat16

_NC_CACHE = None


def _build_nc(repeat=1, phases="abc", collective=True, overlap_c=True,
              filler_gap=5):
    nc = bacc.Bacc("TRN2", target_bir_lowering=False, debug=False,
                   num_devices=N_CORES)

    xq = nc.dram_tensor("xq", [D, S], BF16, kind="ExternalInput")
    xk = nc.dram_tensor("xk", [D, S], BF16, kind="ExternalInput")
    xv = nc.dram_tensor("xv", [D, S], BF16, kind="ExternalInput")
    wqt = nc.dram_tensor("wqt", [D, DL], BF16, kind="ExternalInput")
    wkt = nc.dram_tensor("wkt", [D, DL], BF16, kind="ExternalInput")
    wvt = nc.dram_tensor("wvt", [D, DL], BF16, kind="ExternalInput")
    wot = nc.dram_tensor("wot", [DL, D], BF16, kind="ExternalInput")
    y = nc.dram_tensor("y", [S // 2, D], F32, kind="ExternalOutput")

    ypart = nc.dram_tensor("ypart", [S, D], F32)
    yrs = nc.dram_tensor("yrs", [S // 2, D], F32)

    with tile.TileContext(nc) as tc:
        with (
            tc.tile_pool(name="xp", bufs=20) as xp,          # x input chunks
            tc.tile_pool(name="kqa", bufs=12) as kqa,        # khT/qhT/attn
            tc.tile_pool(name="wp", bufs=3) as wpool,        # wq/wk/wv
            tc.tile_pool(name="wop", bufs=1) as wopool,      # woT
            tc.tile_pool(name="vhp", bufs=16) as vhp,        # vh | ones
            tc.tile_pool(name="expp", bufs=4) as expp,       # exp(scores)
            tc.tile_pool(name="pvsp", bufs=2) as pvsp,       # pv psum drain
            tc.tile_pool(name="rcp", bufs=2) as rcp,         # reciprocal row
            tc.tile_pool(name="rbp", bufs=2) as rbp,         # bcast recip
            tc.tile_pool(name="stgp", bufs=3) as stgp,       # psum->dram stg
            tc.tile_pool(name="scp", bufs=2, space="PSUM") as scp,   # 4 bank
            tc.tile_pool(name="pvp", bufs=2, space="PSUM") as pvp,   # 2 bank
            tc.tile_pool(name="prp", bufs=2, space="PSUM") as prp,   # 2 bank
        ):
            for rep in range(repeat):
                pfx = f"r{rep}_"
                # woT load (bf16): [512, 1024] -> [128, 4, 1024]
                wo_sb = wopool.tile([128, 4, D], BF16, tag="wo",
                                    name=f"{pfx}wo_sb")
                nc.sync.dma_start(
                    out=wo_sb[:], in_=wot[:].rearrange("(t p) n -> p t n", p=128)
                )

                # ---------- V projection (all 16 seq blocks) --------------
                w_sb = wpool.tile([128, 8, DL], BF16, tag="w", name=f"{pfx}w_v")
                nc.sync.dma_start(
                    out=w_sb[:], in_=wvt[:].rearrange("(kc p) m -> p kc m", p=128)
                )
                xv_sb = []
                for kc in range(8):
                    xt = xp.tile([128, S], BF16, tag="x", name=f"{pfx}xv_{kc}")
                    nc.sync.dma_start(out=xt[:], in_=xv[kc * 128:(kc + 1) * 128, :])
                    xv_sb.append(xt)
                vh_sb = []
                for st in range(16):
                    acc = prp.tile([128, 512], F32, tag="pr", name=f"{pfx}psv_{st}")
                    for kc in range(8):
                        nc.tensor.matmul(
                            acc[:],
                            xv_sb[kc][:, st * 128:(st + 1) * 128],
                            w_sb[:, kc, :],
                            start=(kc == 0),
                            stop=(kc == 7),
                        )
                    vt = vhp.tile([128, NH, DH + 1], BF16, tag="vh",
                                  name=f"{pfx}vh_{st}")
                    nc.vector.tensor_copy(
                        vt[:, :, 0:DH], acc[:].rearrange("p (h d) -> p h d", d=DH)
                    )
                    nc.vector.memset(vt[:, :, DH:DH + 1], 1.0)
                    vh_sb.append(vt)

                # ---------- K / Q projections: tiles + lazy chunk emitters
                wk_sb = wpool.tile([128, 8, DL], BF16, tag="w", name=f"{pfx}w_k")
                nc.sync.dma_start(
                    out=wk_sb[:], in_=wkt[:].rearrange("(kc p) m -> p kc m", p=128)
                )
                xk_sb = []
                for kc in range(8):
                    xt = xp.tile([128, S], BF16, tag="x", name=f"{pfx}xk_{kc}")
                    nc.sync.dma_start(out=xt[:], in_=xk[kc * 128:(kc + 1) * 128, :])
                    xk_sb.append(xt)
                wq_sb = wpool.tile([128, 8, DL], BF16, tag="w", name=f"{pfx}w_q")
                nc.sync.dma_start(
                    out=wq_sb[:], in_=wqt[:].rearrange("(kc p) m -> p kc m", p=128)
                )
                xq_sb = []
                for kc in range(8):
                    xt = xp.tile([128, S], BF16, tag="x", name=f"{pfx}xq_{kc}")
                    nc.sync.dma_start(out=xt[:], in_=xq[kc * 128:(kc + 1) * 128, :])
                    xq_sb.append(xt)

                khT_sb = [
                    kqa.tile([128, S], BF16, tag="kqa", name=f"{pfx}khT_{t}")
                    for t in range(4)
                ]
                qhT_sb = [
                    kqa.tile([128, S], BF16, tag="kqa", name=f"{pfx}qhT_{t}")
                    for t in range(4)
                ]
                attn_sb = [
                    kqa.tile([128, S], BF16, tag="kqa", name=f"{pfx}attn_{t}")
                    for t in range(4)
                ]

                def proj_chunk(name, w_sb_, x_sb_, dest, mc, nt):
                    # one [128(dl), 512(seq)] output block of K or Q proj
                    acc = prp.tile([128, 512], F32, tag="pr",
                                   name=f"{pfx}ps{name}_{mc}_{nt}")
                    for kc in range(8):
                        nc.tensor.matmul(
                            acc[:],
                            w_sb_[:, kc, mc * 128:(mc + 1) * 128],
                            x_sb_[kc][:, nt * 512:(nt + 1) * 512],
                            start=(kc == 0),
                            stop=(kc == 7),
                        )
                    nc.vector.tensor_copy(dest[:, nt * 512:(nt + 1) * 512], acc[:])

                def outproj_chunk(qb, nt):
                    acc = prp.tile([128, 512], F32, tag="pr",
                                   name=f"{pfx}psy_{qb}_{nt}")
                    for t in range(4):
                        nc.tensor.matmul(
                            acc[:],
                            attn_sb[t][:, qb * 128:(qb + 1) * 128],
                            wo_sb[:, t, nt * 512:(nt + 1) * 512],
                            start=(t == 0),
                            stop=(t == 3),
                        )
                    st = stgp.tile([128, 512], F32, tag="ystg",
                                   name=f"{pfx}sty_{qb}_{nt}")
                    nc.vector.tensor_copy(st[:], acc[:])
                    nc.sync.dma_start(
                        out=ypart[qb * 128:(qb + 1) * 128,
                                  nt * 512:(nt + 1) * 512],
                        in_=st[:],
                    )
                    if nt == 1 and qb % 2 == 1:
                        ch = qb // 2
                        if collective:
                            nc.gpsimd.collective_compute(
                                "ReduceScatter",
                                mybir.AluOpType.add,
                                replica_groups=[[0, 1], [2, 3], [4, 5], [6, 7]],
                                ins=[ypart[256 * ch:256 * (ch + 1), :].opt()],
                                outs=[yrs[128 * ch:128 * (ch + 1), :].opt()],
                            )
                            nc.sync.dma_start(
                                out=y[128 * ch:128 * (ch + 1), :],
                                in_=yrs[128 * ch:128 * (ch + 1), :],
                            )
                        elif ch < 4:
                            nc.sync.dma_start(
                                out=y[256 * ch:256 * (ch + 1), :],
                                in_=ypart[256 * ch:256 * (ch + 1), :],
                            )

                # filler queue: chunks of PE work emitted into attention
                # slack. Keyed so a chain can force-emit its own inputs
                # before it starts (PE is in-order; a filler landing after
                # its consumer would deadlock).
                from collections import OrderedDict
                fillers = OrderedDict()
                for t in range(1, 4):
                    for nt in range(4):
                        fillers[("k", t, nt)] = (
                            proj_chunk, ("k", wk_sb, xk_sb, khT_sb[t], t, nt))
                # Q chunks appended lazily per qc below.

                # K/Q for pair 0, q-chunk 0 emitted eagerly (lead-in)
                for nt in range(4):
                    proj_chunk("k", wk_sb, xk_sb, khT_sb[0], 0, nt)
                proj_chunk("q", wq_sb, xq_sb, qhT_sb[0], 0, 0)
                q_emitted = {(0, 0)}

                def emit_filler():
                    if fillers:
                        _, (fn, args) = fillers.popitem(last=False)
                        fn(*args)
                        return True
                    return False

                def ensure_filler(key):
                    if key in fillers:
                        fn, args = fillers.pop(key)
                        fn(*args)

                if "b" not in phases:
                    continue

                # ---------------- attention ----------------
                for qc in range(NQC):
                    # queue Q projections needed for this qc (and beyond)
                    for t in range(4):
                        if (t, qc) not in q_emitted:
                            q_emitted.add((t, qc))
                            fillers[("q", t, qc)] = (
                                proj_chunk, ("q", wq_sb, xq_sb,
                                             qhT_sb[t], t, qc))
                    for t in range(4):
                        # inputs this chain depends on must be emitted first
                        for nt in range(4):
                            ensure_filler(("k", t, nt))
                        ensure_filler(("q", t, qc))
                        kh = khT_sb[t]
                        qh = qhT_sb[t]
                        qsl = slice(qc * W, (qc + 1) * W)
                        pv = [
                            pvp.tile([DH + 1, W], F32, tag="pv",
                                     name=f"{pfx}pv_{t}_{qc}_{p}")
                            for p in range(2)
                        ]
                        ex_tiles = [None] * 16
                        sc_tiles = [None] * 16
                        for kb in range(16):
                            sc = scp.tile([128, 2 * W], F32, tag="sc",
                                          name=f"{pfx}sc_{t}_{qc}_{kb}")
                            sc_tiles[kb] = sc
                            ksl = slice(kb * 128, (kb + 1) * 128)
                            # two concurrent row-tiled matmuls (tiles T0/T8),
                            # each writing its own PSUM bank of sc
                            for p in range(2):
                                hsl = slice(64 * p, 64 * p + 64)
                                nc.tensor.matmul(
                                    sc[:, p * W:(p + 1) * W],
                                    kh[hsl, ksl],
                                    qh[hsl, qsl],
                                    start=True,
                                    stop=True,
                                )
                            ex = expp.tile([128, 2 * W], BF16, tag="exp",
                                           name=f"{pfx}ex_{t}_{qc}_{kb}")
                            ex_tiles[kb] = ex
                            nc.scalar.activation(
                                ex[:], sc[:], mybir.ActivationFunctionType.Exp,
                                scale=SCALE,
                            )
                            # pv accumulation for previous kb emitted after
                            # this kb's scores to keep ACT double-buffered
                            if kb > 0:
                                for p in range(2):
                                    nc.tensor.matmul(
                                        pv[p][:],
                                        vh_sb[kb - 1][:, 2 * t + p, :],
                                        ex_tiles[kb - 1][:, p * W:(p + 1) * W],
                                        start=(kb - 1 == 0),
                                        stop=False,
                                    )
                            if kb % filler_gap == filler_gap - 1:
                                emit_filler()
                        for p in range(2):
                            nc.tensor.matmul(
                                pv[p][:],
                                vh_sb[15][:, 2 * t + p, :],
                                ex_tiles[15][:, p * W:(p + 1) * W],
                                start=False,
                                stop=True,
                            )
                        # normalize: out rows 0..63 divided by ones-row 64
                        for p in range(2):
                            pvs = pvsp.tile([DH + 1, W], F32, tag="pvs",
                                            name=f"{pfx}pvs_{t}_{qc}_{p}")
                            nc.vector.tensor_copy(pvs[:], pv[p][:])
                            rc = rcp.tile([1, W], F32, tag="rc",
                                          name=f"{pfx}rc_{t}_{qc}_{p}")
                            nc.vector.reciprocal(rc[:], pvs[DH:DH + 1, :])
                            rb = rbp.tile([DH, W], F32, tag="rb",
                                          name=f"{pfx}rb_{t}_{qc}_{p}")
                            nc.gpsimd.partition_broadcast(rb[:], rc[:])
                            nc.vector.tensor_mul(
                                attn_sb[t][64 * p:64 * p + 64, qsl],
                                pvs[0:DH, :], rb[:]
                            )
                    # output projection for this qc's 4 row-blocks
                    if "c" in phases:
                        if qc < NQC - 1:
                            for qb in range(4 * qc, 4 * qc + 4):
                                for nt in range(2):
                                    fillers[("y", qb, nt)] = (
                                        outproj_chunk, (qb, nt))
                        else:
                            # drain all remaining fillers, then final outproj
                            while emit_filler():
                                pass
                            for qb in range(4 * qc, 4 * qc + 4):
                                for nt in range(2):
                                    outproj_chunk(qb, nt)

    nc.finalize()
    return nc


def _get_nc():
    global _NC_CACHE
    if _NC_CACHE is None:
        _NC_CACHE = _build_nc()
    return _NC_CACHE


def kernel(q, k, v, wq, wk, wv, wo, _res_hook=None):
    q = np.asarray(q, dtype=np.float32)
    k = np.asarray(k, dtype=np.float32)
    v = np.asarray(v, dtype=np.float32)
    wq = np.asarray(wq, dtype=np.float32)
    wk = np.asarray(wk, dtype=np.float32)
    wv = np.asarray(wv, dtype=np.float32)
    wo = np.asarray(wo, dtype=np.float32)
    B = q.shape[0]

    nc = _get_nc()
    in_maps = []
    for c in range(N_CORES):
        b, g = c // 2, c % 2
        sl = slice(DL * g, DL * (g + 1))
        in_maps.append({
            "xq": np.ascontiguousarray(q[b].T).astype(ml_dtypes.bfloat16),
            "xk": np.ascontiguousarray(k[b].T).astype(ml_dtypes.bfloat16),
            "xv": np.ascontiguousarray(v[b].T).astype(ml_dtypes.bfloat16),
            "wqt": np.ascontiguousarray(wq[sl, :].T).astype(ml_dtypes.bfloat16),
            "wkt": np.ascontiguousarray(wk[sl, :].T).astype(ml_dtypes.bfloat16),
            "wvt": np.ascontiguousarray(wv[sl, :].T).astype(ml_dtypes.bfloat16),
            "wot": np.ascontiguousarray(wo[:, sl].T).astype(ml_dtypes.bfloat16),
        })

    res = run_bass_kernel_spmd(nc, in_maps, list(range(N_CORES)))
    if _res_hook is not None:
        _res_hook(res)

    out = np.empty((B, S, D), dtype=np.float32)
    for c in range(N_CORES):
        b, g = c // 2, c % 2
        yc = res.results[c]["y"]
        for ch in range(8):
            out[b, 256 * ch + 128 * g:256 * ch + 128 * (g + 1), :] = \
                yc[128 * ch:128 * (ch + 1), :]
    return out


# revision 15
# speedup vs baseline: 5.8568x; 5.8568x over previous
"""Multi-head attention (B=4, S=2048, D=1024, H=16) on 8 TRN2 NeuronCores.

Sharding: core c = 2*b + g handles batch b (of 4) and head-group g (of 2,
8 heads / 512 model dims each).  Per core (all matmuls bf16, fp32 PSUM):
  - QKV projections for its batch restricted to its 512 output dims;
    qhT/khT [512, 2048] and vh [2048, 520] stay resident in SBUF
  - attention for its 8 heads in transposed-scores layout (scoresT[k, q]):
    softmax denominator via a ones-column appended to V; no max subtraction
    (scores are ~N(0, 0.08^2) after the 1/32 scale, exp cannot overflow)
  - structured to keep ScalarE (exp) saturated: per (head-pair, 512-wide
    q chunk) the two heads' scores land in one [128, 1024] PSUM tile
    (different banks, written by two concurrent 64x128 row-tiled matmuls)
    so one ACT instruction covers both heads; sc double-buffered; K/Q
    projections for later pairs and the output projection are emitted as
    "filler" chunks inside the attention kb loop to fill PE slack
  - output projection partial over its 512 model dims; partials
    ReduceScatter'd pairwise in 8 chunks so the collective overlaps compute
Host: pre-transposes inputs/weights (bf16), feeds per-core shards, and
reassembles the full [4, 2048, 1024] fp32 output from the 8 per-core
outputs (chunked-RS row interleaving: core 2b+g holds rows
256*ch + [128*g, 128*(g+1)) of batch b for ch in 0..7).
"""

import numpy as np
import ml_dtypes

import concourse.bass as bass
import concourse.mybir as mybir
import concourse.tile as tile
from concourse import bacc
from concourse.bass_utils import run_bass_kernel_spmd

N_CORES = 8
S = 2048          # sequence length
D = 1024          # d_model
DL = 512          # local model dims (8 heads x 64)
NH = 8            # local heads
DH = 64           # head dim
W = 512           # q-chunk width for attention
NQC = S // W      # 4 q chunks
SCALE = 1.0 / 32.0  # 1/sqrt(d_model)

F32 = mybir.dt.float32
BF16 = mybir.dt.bfloat16

_NC_CACHE = None


def _build_nc(repeat=1, phases="abc", collective=True, overlap_c=True,
              filler_gap=2):
    nc = bacc.Bacc("TRN2", target_bir_lowering=False, debug=False,
                   num_devices=N_CORES)

    xq = nc.dram_tensor("xq", [D, S], BF16, kind="ExternalInput")
    xk = nc.dram_tensor("xk", [D, S], BF16, kind="ExternalInput")
    xv = nc.dram_tensor("xv", [D, S], BF16, kind="ExternalInput")
    wqt = nc.dram_tensor("wqt", [D, DL], BF16, kind="ExternalInput")
    wkt = nc.dram_tensor("wkt", [D, DL], BF16, kind="ExternalInput")
    wvt = nc.dram_tensor("wvt", [D, DL], BF16, kind="ExternalInput")
    wot = nc.dram_tensor("wot", [DL, D], BF16, kind="ExternalInput")
    y = nc.dram_tensor("y", [S // 2, D], F32, kind="ExternalOutput")

    ypart = nc.dram_tensor("ypart", [S, D], F32)
    yrs = nc.dram_tensor("yrs", [S // 2, D], F32)

    with tile.TileContext(nc) as tc:
        with (
            tc.tile_pool(name="xp", bufs=20) as xp,          # x input chunks
            tc.tile_pool(name="kqa", bufs=12) as kqa,        # khT/qhT/attn
            tc.tile_pool(name="wp", bufs=3) as wpool,        # wq/wk/wv
            tc.tile_pool(name="wop", bufs=1) as wopool,      # woT
            tc.tile_pool(name="vhp", bufs=16) as vhp,        # vh | ones
            tc.tile_pool(name="expp", bufs=4) as expp,       # exp(scores)
            tc.tile_pool(name="pvsp", bufs=2) as pvsp,       # pv psum drain
            tc.tile_pool(name="rcp", bufs=2) as rcp,         # reciprocal row
            tc.tile_pool(name="rbp", bufs=2) as rbp,         # bcast recip
            tc.tile_pool(name="stgp", bufs=3) as stgp,       # psum->dram stg
            tc.tile_pool(name="scp", bufs=2, space="PSUM") as scp,   # 4 bank
            tc.tile_pool(name="pvp", bufs=2, space="PSUM") as pvp,   # 2 bank
            tc.tile_pool(name="prp", bufs=2, space="PSUM") as prp,   # 2 bank
        ):
            for rep in range(repeat):
                pfx = f"r{rep}_"
                # woT load (bf16): [512, 1024] -> [128, 4, 1024]
                wo_sb = wopool.tile([128, 4, D], BF16, tag="wo",
                                    name=f"{pfx}wo_sb")
                nc.sync.dma_start(
                    out=wo_sb[:], in_=wot[:].rearrange("(t p) n -> p t n", p=128)
                )

                # ---------- V projection (all 16 seq blocks) --------------
                w_sb = wpool.tile([128, 8, DL], BF16, tag="w", name=f"{pfx}w_v")
                nc.sync.dma_start(
                    out=w_sb[:], in_=wvt[:].rearrange("(kc p) m -> p kc m", p=128)
                )
                xv_sb = []
                for kc in range(8):
                    xt = xp.tile([128, S], BF16, tag="x", name=f"{pfx}xv_{kc}")
                    nc.sync.dma_start(out=xt[:], in_=xv[kc * 128:(kc + 1) * 128, :])
                    xv_sb.append(xt)
                vh_sb = []
                for st in range(16):
                    acc = prp.tile([128, 512], F32, tag="pr", name=f"{pfx}psv_{st}")
                    for kc in range(8):
                        nc.tensor.matmul(
                            acc[:],
                            xv_sb[kc][:, st * 128:(st + 1) * 128],
                            w_sb[:, kc, :],
                            start=(kc == 0),
                            stop=(kc == 7),
                        )
                    vt = vhp.tile([128, NH, DH + 1], BF16, tag="vh",
                                  name=f"{pfx}vh_{st}")
                    nc.vector.tensor_copy(
                        vt[:, :, 0:DH], acc[:].rearrange("p (h d) -> p h d", d=DH)
                    )
                    nc.vector.memset(vt[:, :, DH:DH + 1], 1.0)
                    vh_sb.append(vt)

                # ---------- K / Q projections: tiles + lazy chunk emitters
                wk_sb = wpool.tile([128, 8, DL], BF16, tag="w", name=f"{pfx}w_k")
                nc.sync.dma_start(
                    out=wk_sb[:], in_=wkt[:].rearrange("(kc p) m -> p kc m", p=128)
                )
                xk_sb = []
                for kc in range(8):
                    xt = xp.tile([128, S], BF16, tag="x", name=f"{pfx}xk_{kc}")
                    nc.sync.dma_start(out=xt[:], in_=xk[kc * 128:(kc + 1) * 128, :])
                    xk_sb.append(xt)
                wq_sb = wpool.tile([128, 8, DL], BF16, tag="w", name=f"{pfx}w_q")
                nc.sync.dma_start(
                    out=wq_sb[:], in_=wqt[:].rearrange("(kc p) m -> p kc m", p=128)
                )
                xq_sb = []
                for kc in range(8):
                    xt = xp.tile([128, S], BF16, tag="x", name=f"{pfx}xq_{kc}")
                    nc.sync.dma_start(out=xt[:], in_=xq[kc * 128:(kc + 1) * 128, :])
                    xq_sb.append(xt)

                khT_sb = [
                    kqa.tile([128, S], BF16, tag="kqa", name=f"{pfx}khT_{t}")
                    for t in range(4)
                ]
                qhT_sb = [
                    kqa.tile([128, S], BF16, tag="kqa", name=f"{pfx}qhT_{t}")
                    for t in range(4)
                ]
                attn_sb = [
                    kqa.tile([128, S], BF16, tag="kqa", name=f"{pfx}attn_{t}")
                    for t in range(4)
                ]

                proj_accs = {}

                def proj_half(name, w_sb_, x_sb_, dest, mc, nt, half):
                    # half of one [128(dl), 512(seq)] block of K or Q proj;
                    # the PSUM accumulator persists between the two halves
                    # so each filler insertion stays within PE slack
                    if half == 0:
                        acc = prp.tile([128, 512], F32, tag="pr",
                                       name=f"{pfx}ps{name}_{mc}_{nt}")
                        proj_accs[(name, mc, nt)] = acc
                    else:
                        acc = proj_accs.pop((name, mc, nt))
                    for kc in range(4 * half, 4 * half + 4):
                        nc.tensor.matmul(
                            acc[:],
                            w_sb_[:, kc, mc * 128:(mc + 1) * 128],
                            x_sb_[kc][:, nt * 512:(nt + 1) * 512],
                            start=(kc == 0),
                            stop=(kc == 7),
                        )
                    if half == 1:
                        nc.vector.tensor_copy(
                            dest[:, nt * 512:(nt + 1) * 512], acc[:])

                def proj_chunk(name, w_sb_, x_sb_, dest, mc, nt):
                    proj_half(name, w_sb_, x_sb_, dest, mc, nt, 0)
                    proj_half(name, w_sb_, x_sb_, dest, mc, nt, 1)

                def outproj_half(qb, nt, half):
                    if half == 0:
                        acc = prp.tile([128, 512], F32, tag="pr",
                                       name=f"{pfx}psy_{qb}_{nt}")
                        proj_accs[("y", qb, nt)] = acc
                    else:
                        acc = proj_accs.pop(("y", qb, nt))
                    for t in (2 * half, 2 * half + 1):
                        nc.tensor.matmul(
                            acc[:],
                            attn_sb[t][:, qb * 128:(qb + 1) * 128],
                            wo_sb[:, t, nt * 512:(nt + 1) * 512],
                            start=(t == 0),
                            stop=(t == 3),
                        )
                    if half == 0:
                        return
                    st = stgp.tile([128, 512], F32, tag="ystg",
                                   name=f"{pfx}sty_{qb}_{nt}")
                    nc.vector.tensor_copy(st[:], acc[:])
                    nc.sync.dma_start(
                        out=ypart[qb * 128:(qb + 1) * 128,
                                  nt * 512:(nt + 1) * 512],
                        in_=st[:],
                    )
                    if nt == 1 and qb % 2 == 1:
                        ch = qb // 2
                        if collective:
                            nc.gpsimd.collective_compute(
                                "ReduceScatter",
                                mybir.AluOpType.add,
                                replica_groups=[[0, 1], [2, 3], [4, 5], [6, 7]],
                                ins=[ypart[256 * ch:256 * (ch + 1), :].opt()],
                                outs=[yrs[128 * ch:128 * (ch + 1), :].opt()],
                            )
                            nc.sync.dma_start(
                                out=y[128 * ch:128 * (ch + 1), :],
                                in_=yrs[128 * ch:128 * (ch + 1), :],
                            )
                        elif ch < 4:
                            nc.sync.dma_start(
                                out=y[256 * ch:256 * (ch + 1), :],
                                in_=ypart[256 * ch:256 * (ch + 1), :],
                            )

                # filler queue: 4-matmul half-chunks of PE work emitted into
                # attention slack, strictly FIFO (PSUM accumulators persist
                # between a chunk's two halves, so pops must stay in order).
                from collections import OrderedDict
                fillers = OrderedDict()
                for t in range(1, 4):
                    for nt in range(4):
                        for h in range(2):
                            fillers[("k", t, nt, h)] = (
                                proj_half,
                                ("k", wk_sb, xk_sb, khT_sb[t], t, nt, h))
                    # pair t's first-qc Q right after its K so the
                    # chain-start ensure drains only what it needs
                    for h in range(2):
                        fillers[("q", t, 0, h)] = (
                            proj_half, ("q", wq_sb, xq_sb,
                                        qhT_sb[t], t, 0, h))

                # K/Q for pair 0, q-chunk 0 emitted eagerly (lead-in)
                for nt in range(4):
                    proj_chunk("k", wk_sb, xk_sb, khT_sb[0], 0, nt)
                proj_chunk("q", wq_sb, xq_sb, qhT_sb[0], 0, 0)
                q_emitted = {(0, 0), (1, 0), (2, 0), (3, 0)}

                def emit_filler():
                    if fillers:
                        _, (fn, args) = fillers.popitem(last=False)
                        fn(*args)
                        return True
                    return False

                def ensure_filler(key):
                    # drain the queue in FIFO order until key is emitted
                    while key in fillers:
                        emit_filler()

                if "b" not in phases:
                    continue

                # ---------------- attention ----------------
                for qc in range(NQC):
                    # queue Q projections needed for this qc (and beyond)
                    for t in range(4):
                        if (t, qc) not in q_emitted:
                            q_emitted.add((t, qc))
                            for h in range(2):
                                fillers[("q", t, qc, h)] = (
                                    proj_half, ("q", wq_sb, xq_sb,
                                                qhT_sb[t], t, qc, h))
                    for t in range(4):
                        # inputs this chain depends on must be emitted first
                        ensure_filler(("k", t, 3, 1))
                        ensure_filler(("q", t, qc, 1))
                        kh = khT_sb[t]
                        qh = qhT_sb[t]
                        qsl = slice(qc * W, (qc + 1) * W)
                        pv = [
                            pvp.tile([DH + 1, W], F32, tag="pv",
                                     name=f"{pfx}pv_{t}_{qc}_{p}")
                            for p in range(2)
                        ]
                        ex_tiles = [None] * 16
                        sc_tiles = [None] * 16
                        for kb in range(16):
                            sc = scp.tile([128, 2 * W], F32, tag="sc",
                                          name=f"{pfx}sc_{t}_{qc}_{kb}")
                            sc_tiles[kb] = sc
                            ksl = slice(kb * 128, (kb + 1) * 128)
                            # two concurrent row-tiled matmuls (tiles T0/T8),
                            # each writing its own PSUM bank of sc
                            for p in range(2):
                                hsl = slice(64 * p, 64 * p + 64)
                                nc.tensor.matmul(
                                    sc[:, p * W:(p + 1) * W],
                                    kh[hsl, ksl],
                                    qh[hsl, qsl],
                                    start=True,
                                    stop=True,
                                )
                            ex = expp.tile([128, 2 * W], BF16, tag="exp",
                                           name=f"{pfx}ex_{t}_{qc}_{kb}")
                            ex_tiles[kb] = ex
                            nc.scalar.activation(
                                ex[:], sc[:], mybir.ActivationFunctionType.Exp,
                                scale=SCALE,
                            )
                            # pv accumulation for previous kb emitted after
                            # this kb's scores to keep ACT double-buffered
                            if kb > 0:
                                for p in range(2):
                                    nc.tensor.matmul(
                                        pv[p][:],
                                        vh_sb[kb - 1][:, 2 * t + p, :],
                                        ex_tiles[kb - 1][:, p * W:(p + 1) * W],
                                        start=(kb - 1 == 0),
                                        stop=False,
                                    )
                            if kb % filler_gap == filler_gap - 1:
                                emit_filler()
                        for p in range(2):
                            nc.tensor.matmul(
                                pv[p][:],
                                vh_sb[15][:, 2 * t + p, :],
                                ex_tiles[15][:, p * W:(p + 1) * W],
                                start=False,
                                stop=True,
                            )
                        # normalize: out rows 0..63 divided by ones-row 64
                        for p in range(2):
                            pvs = pvsp.tile([DH + 1, W], F32, tag="pvs",
                                            name=f"{pfx}pvs_{t}_{qc}_{p}")
                            nc.vector.tensor_copy(pvs[:], pv[p][:])
                            rc = rcp.tile([1, W], F32, tag="rc",
                                          name=f"{pfx}rc_{t}_{qc}_{p}")
                            nc.vector.reciprocal(rc[:], pvs[DH:DH + 1, :])
                            rb = rbp.tile([DH, W], F32, tag="rb",
                                          name=f"{pfx}rb_{t}_{qc}_{p}")
                            nc.gpsimd.partition_broadcast(rb[:], rc[:])
                            nc.vector.tensor_mul(
                                attn_sb[t][64 * p:64 * p + 64, qsl],
                                pvs[0:DH, :], rb[:]
                            )
                    # output projection for this qc's 4 row-blocks
                    if "c" in phases:
                        if qc < NQC - 1:
                            for qb in range(4 * qc, 4 * qc + 4):
                                for nt in range(2):
                                    for h in range(2):
                                        fillers[("y", qb, nt, h)] = (
                                            outproj_half, (qb, nt, h))
                        else:
                            # drain all remaining fillers, then final outproj
                            while emit_filler():
                                pass
                            for qb in range(4 * qc, 4 * qc + 4):
                                for nt in range(2):
                                    outproj_half(qb, nt, 0)
                                    outproj_half(qb, nt, 1)

    nc.finalize()
    return nc


def _get_nc():
    global _NC_CACHE
    if _NC_CACHE is None:
        _NC_CACHE = _build_nc()
    return _NC_CACHE


def kernel(q, k, v, wq, wk, wv, wo, _res_hook=None):
    q = np.asarray(q, dtype=np.float32)
    k = np.asarray(k, dtype=np.float32)
    v = np.asarray(v, dtype=np.float32)
    wq = np.asarray(wq, dtype=np.float32)
    wk = np.asarray(wk, dtype=np.float32)
    wv = np.asarray(wv, dtype=np.float32)
    wo = np.asarray(wo, dtype=np.float32)
    B = q.shape[0]

    nc = _get_nc()
    in_maps = []
    for c in range(N_CORES):
        b, g = c // 2, c % 2
        sl = slice(DL * g, DL * (g + 1))
        in_maps.append({
            "xq": np.ascontiguousarray(q[b].T).astype(ml_dtypes.bfloat16),
            "xk": np.ascontiguousarray(k[b].T).astype(ml_dtypes.bfloat16),
            "xv": np.ascontiguousarray(v[b].T).astype(ml_dtypes.bfloat16),
            "wqt": np.ascontiguousarray(wq[sl, :].T).astype(ml_dtypes.bfloat16),
            "wkt": np.ascontiguousarray(wk[sl, :].T).astype(ml_dtypes.bfloat16),
            "wvt": np.ascontiguousarray(wv[sl, :].T).astype(ml_dtypes.bfloat16),
            "wot": np.ascontiguousarray(wo[:, sl].T).astype(ml_dtypes.bfloat16),
        })

    res = run_bass_kernel_spmd(nc, in_maps, list(range(N_CORES)))
    if _res_hook is not None:
        _res_hook(res)

    out = np.empty((B, S, D), dtype=np.float32)
    for c in range(N_CORES):
        b, g = c // 2, c % 2
        yc = res.results[c]["y"]
        for ch in range(8):
            out[b, 256 * ch + 128 * g:256 * ch + 128 * (g + 1), :] = \
                yc[128 * ch:128 * (ch + 1), :]
    return out


# revision 20
# speedup vs baseline: 7.7989x; 1.3316x over previous
"""Multi-head attention (B=4, S=2048, D=1024, H=16) on 8 TRN2 NeuronCores.

Sharding: core c = 2*b + g handles batch b (of 4) and head-group g (of 2,
8 heads / 512 model dims each).  Per core (all matmuls bf16, fp32 PSUM):
  - QKV projections for its batch restricted to its 512 output dims;
    qhT/khT [512, 2048] and vh [2048, 520] stay resident in SBUF
  - attention for its 8 heads in transposed-scores layout (scoresT[k, q]):
    softmax denominator via a ones-column appended to V; no max subtraction
    (scores are ~N(0, 0.08^2) after the 1/32 scale, exp cannot overflow)
  - structured to keep ScalarE (exp) saturated: per (head-pair, 512-wide
    q chunk) the two heads' scores land in one [128, 1024] PSUM tile
    (different banks, written by two concurrent 64x128 row-tiled matmuls)
    so one ACT instruction covers both heads; sc double-buffered; K/Q
    projections for later pairs and the output projection are emitted as
    "filler" chunks inside the attention kb loop to fill PE slack
  - output projection partial over its 512 model dims; partials
    ReduceScatter'd pairwise in 8 chunks so the collective overlaps compute
Host: pre-transposes inputs/weights (bf16), feeds per-core shards, and
reassembles the full [4, 2048, 1024] fp32 output from the 8 per-core
outputs (chunked-RS row interleaving: core 2b+g holds rows
256*ch + [128*g, 128*(g+1)) of batch b for ch in 0..7).
"""

import numpy as np
import ml_dtypes

import concourse.bass as bass
import concourse.mybir as mybir
import concourse.tile as tile
from concourse import bacc
from concourse.bass_utils import run_bass_kernel_spmd

N_CORES = 8
S = 2048          # sequence length
D = 1024          # d_model
DL = 512          # local model dims (8 heads x 64)
NH = 8            # local heads
DH = 64           # head dim
W = 512           # q-chunk width for attention
NQC = S // W      # 4 q chunks
SCALE = 1.0 / 32.0  # 1/sqrt(d_model)

F32 = mybir.dt.float32
BF16 = mybir.dt.bfloat16

_NC_CACHE = None


def _build_nc(repeat=1, phases="abc", collective=True, overlap_c=True,
              filler_gap=2):
    nc = bacc.Bacc("TRN2", target_bir_lowering=False, debug=False,
                   num_devices=N_CORES)

    xq = nc.dram_tensor("xq", [D, S], BF16, kind="ExternalInput")
    xk = nc.dram_tensor("xk", [D, S], BF16, kind="ExternalInput")
    xv = nc.dram_tensor("xv", [D, S], BF16, kind="ExternalInput")
    wqt = nc.dram_tensor("wqt", [D, DL], BF16, kind="ExternalInput")
    wkt = nc.dram_tensor("wkt", [D, DL], BF16, kind="ExternalInput")
    wvt = nc.dram_tensor("wvt", [D, DL], BF16, kind="ExternalInput")
    wot = nc.dram_tensor("wot", [DL, D], BF16, kind="ExternalInput")
    y = nc.dram_tensor("y", [S // 2, D], F32, kind="ExternalOutput")

    ypart = nc.dram_tensor("ypart", [S, D], F32)
    yrs = nc.dram_tensor("yrs", [S // 2, D], F32)

    with tile.TileContext(nc) as tc:
        with (
            tc.tile_pool(name="xp", bufs=20) as xp,          # x input chunks
            tc.tile_pool(name="kqa", bufs=12) as kqa,        # khT/qhT/attn
            tc.tile_pool(name="wp", bufs=3) as wpool,        # wq/wk/wv
            tc.tile_pool(name="wop", bufs=1) as wopool,      # woT
            tc.tile_pool(name="vhp", bufs=16) as vhp,        # vh | ones
            tc.tile_pool(name="expp", bufs=4) as expp,       # exp(scores)
            tc.tile_pool(name="pvsp", bufs=2) as pvsp,       # pv psum drain
            tc.tile_pool(name="rcp", bufs=2) as rcp,         # reciprocal row
            tc.tile_pool(name="rbp", bufs=2) as rbp,         # bcast recip
            tc.tile_pool(name="stgp", bufs=3) as stgp,       # psum->dram stg
            tc.tile_pool(name="scp", bufs=2, space="PSUM") as scp,   # 4 bank
            tc.tile_pool(name="pvp", bufs=2, space="PSUM") as pvp,   # 2 bank
            tc.tile_pool(name="prp", bufs=2, space="PSUM") as prp,   # 2 bank
        ):
            for rep in range(repeat):
                pfx = f"r{rep}_"
                # woT load (bf16): [512, 1024] -> [128, 4, 1024]
                wo_sb = wopool.tile([128, 4, D], BF16, tag="wo",
                                    name=f"{pfx}wo_sb")
                nc.sync.dma_start(
                    out=wo_sb[:], in_=wot[:].rearrange("(t p) n -> p t n", p=128)
                )

                # ---------- V projection (all 16 seq blocks) --------------
                w_sb = wpool.tile([128, 8, DL], BF16, tag="w", name=f"{pfx}w_v")
                nc.sync.dma_start(
                    out=w_sb[:], in_=wvt[:].rearrange("(kc p) m -> p kc m", p=128)
                )
                xv_sb = []
                for kc in range(8):
                    xt = xp.tile([128, S], BF16, tag="x", name=f"{pfx}xv_{kc}")
                    nc.sync.dma_start(out=xt[:], in_=xv[kc * 128:(kc + 1) * 128, :])
                    xv_sb.append(xt)
                vh_sb = []
                for st in range(16):
                    acc = prp.tile([128, 512], F32, tag="pr", name=f"{pfx}psv_{st}")
                    for kc in range(8):
                        nc.tensor.matmul(
                            acc[:],
                            xv_sb[kc][:, st * 128:(st + 1) * 128],
                            w_sb[:, kc, :],
                            start=(kc == 0),
                            stop=(kc == 7),
                        )
                    vt = vhp.tile([128, NH, DH + 1], BF16, tag="vh",
                                  name=f"{pfx}vh_{st}")
                    nc.vector.tensor_copy(
                        vt[:, :, 0:DH], acc[:].rearrange("p (h d) -> p h d", d=DH)
                    )
                    nc.vector.memset(vt[:, :, DH:DH + 1], 1.0)
                    vh_sb.append(vt)

                # ---------- K / Q projections: tiles + lazy chunk emitters
                wk_sb = wpool.tile([128, 8, DL], BF16, tag="w", name=f"{pfx}w_k")
                nc.sync.dma_start(
                    out=wk_sb[:], in_=wkt[:].rearrange("(kc p) m -> p kc m", p=128)
                )
                xk_sb = []
                for kc in range(8):
                    xt = xp.tile([128, S], BF16, tag="x", name=f"{pfx}xk_{kc}")
                    nc.sync.dma_start(out=xt[:], in_=xk[kc * 128:(kc + 1) * 128, :])
                    xk_sb.append(xt)
                wq_sb = wpool.tile([128, 8, DL], BF16, tag="w", name=f"{pfx}w_q")
                nc.sync.dma_start(
                    out=wq_sb[:], in_=wqt[:].rearrange("(kc p) m -> p kc m", p=128)
                )
                xq_sb = []
                for kc in range(8):
                    xt = xp.tile([128, S], BF16, tag="x", name=f"{pfx}xq_{kc}")
                    nc.sync.dma_start(out=xt[:], in_=xq[kc * 128:(kc + 1) * 128, :])
                    xq_sb.append(xt)

                khT_sb = [
                    kqa.tile([128, S], BF16, tag="kqa", name=f"{pfx}khT_{t}")
                    for t in range(4)
                ]
                qhT_sb = [
                    kqa.tile([128, S], BF16, tag="kqa", name=f"{pfx}qhT_{t}")
                    for t in range(4)
                ]
                attn_sb = [
                    kqa.tile([128, S], BF16, tag="kqa", name=f"{pfx}attn_{t}")
                    for t in range(4)
                ]

                proj_accs = {}

                def proj_half(name, w_sb_, x_sb_, dest, mc, nt, half):
                    # half of one [128(dl), 512(seq)] block of K or Q proj;
                    # the PSUM accumulator persists between the two halves
                    # so each filler insertion stays within PE slack
                    if half == 0:
                        acc = prp.tile([128, 512], F32, tag="pr",
                                       name=f"{pfx}ps{name}_{mc}_{nt}")
                        proj_accs[(name, mc, nt)] = acc
                    else:
                        acc = proj_accs.pop((name, mc, nt))
                    for kc in range(4 * half, 4 * half + 4):
                        nc.tensor.matmul(
                            acc[:],
                            w_sb_[:, kc, mc * 128:(mc + 1) * 128],
                            x_sb_[kc][:, nt * 512:(nt + 1) * 512],
                            start=(kc == 0),
                            stop=(kc == 7),
                        )
                    if half == 1:
                        nc.vector.tensor_copy(
                            dest[:, nt * 512:(nt + 1) * 512], acc[:])

                def proj_chunk(name, w_sb_, x_sb_, dest, mc, nt):
                    proj_half(name, w_sb_, x_sb_, dest, mc, nt, 0)
                    proj_half(name, w_sb_, x_sb_, dest, mc, nt, 1)

                def outproj_half(qb, nt, half):
                    if half == 0:
                        acc = prp.tile([128, 512], F32, tag="pr",
                                       name=f"{pfx}psy_{qb}_{nt}")
                        proj_accs[("y", qb, nt)] = acc
                    else:
                        acc = proj_accs.pop(("y", qb, nt))
                    for t in (2 * half, 2 * half + 1):
                        nc.tensor.matmul(
                            acc[:],
                            attn_sb[t][:, qb * 128:(qb + 1) * 128],
                            wo_sb[:, t, nt * 512:(nt + 1) * 512],
                            start=(t == 0),
                            stop=(t == 3),
                        )
                    if half == 0:
                        return
                    st = stgp.tile([128, 512], F32, tag="ystg",
                                   name=f"{pfx}sty_{qb}_{nt}")
                    nc.vector.tensor_copy(st[:], acc[:])
                    nc.sync.dma_start(
                        out=ypart[qb * 128:(qb + 1) * 128,
                                  nt * 512:(nt + 1) * 512],
                        in_=st[:],
                    )
                    if nt == 1 and qb % 2 == 1:
                        ch = qb // 2
                        if collective:
                            nc.gpsimd.collective_compute(
                                "ReduceScatter",
                                mybir.AluOpType.add,
                                replica_groups=[[0, 1], [2, 3], [4, 5], [6, 7]],
                                ins=[ypart[256 * ch:256 * (ch + 1), :].opt()],
                                outs=[yrs[128 * ch:128 * (ch + 1), :].opt()],
                            )
                            nc.sync.dma_start(
                                out=y[128 * ch:128 * (ch + 1), :],
                                in_=yrs[128 * ch:128 * (ch + 1), :],
                            )
                        elif ch < 4:
                            nc.sync.dma_start(
                                out=y[256 * ch:256 * (ch + 1), :],
                                in_=ypart[256 * ch:256 * (ch + 1), :],
                            )

                # filler queue: 4-matmul half-chunks of PE work emitted into
                # attention slack, strictly FIFO (PSUM accumulators persist
                # between a chunk's two halves, so pops must stay in order).
                from collections import OrderedDict
                fillers = OrderedDict()
                for t in range(1, 4):
                    for nt in range(4):
                        for h in range(2):
                            fillers[("k", t, nt, h)] = (
                                proj_half,
                                ("k", wk_sb, xk_sb, khT_sb[t], t, nt, h))
                    # pair t's first-qc Q right after its K so the
                    # chain-start ensure drains only what it needs
                    for h in range(2):
                        fillers[("q", t, 0, h)] = (
                            proj_half, ("q", wq_sb, xq_sb,
                                        qhT_sb[t], t, 0, h))

                # K/Q for pair 0, q-chunk 0 emitted eagerly (lead-in)
                for nt in range(4):
                    proj_chunk("k", wk_sb, xk_sb, khT_sb[0], 0, nt)
                proj_chunk("q", wq_sb, xq_sb, qhT_sb[0], 0, 0)
                q_emitted = {(0, 0), (1, 0), (2, 0), (3, 0)}

                def emit_filler():
                    if fillers:
                        _, (fn, args) = fillers.popitem(last=False)
                        fn(*args)
                        return True
                    return False

                def ensure_filler(key):
                    # drain the queue in FIFO order until key is emitted
                    while key in fillers:
                        emit_filler()

                if "b" not in phases:
                    continue

                # ---------------- attention ----------------
                def make_tail(t, qc, pv, ex15):
                    # last pv accumulation + softmax normalize of a chain,
                    # deferred into the next chain's first kb so the next
                    # chain's first scores matmul (and its ACT) aren't
                    # queued behind it on the in-order PE
                    qsl = slice(qc * W, (qc + 1) * W)

                    def tail():
                        for p in range(2):
                            nc.tensor.matmul(
                                pv[p][:],
                                vh_sb[15][:, 2 * t + p, :],
                                ex15[:, p * W:(p + 1) * W],
                                start=False,
                                stop=True,
                            )
                        for p in range(2):
                            pvs = pvsp.tile([DH + 1, W], F32, tag="pvs",
                                            name=f"{pfx}pvs_{t}_{qc}_{p}")
                            nc.vector.tensor_copy(pvs[:], pv[p][:])
                            rc = rcp.tile([1, W], F32, tag="rc",
                                          name=f"{pfx}rc_{t}_{qc}_{p}")
                            nc.vector.reciprocal(rc[:], pvs[DH:DH + 1, :])
                            rb = rbp.tile([DH, W], F32, tag="rb",
                                          name=f"{pfx}rb_{t}_{qc}_{p}")
                            nc.gpsimd.partition_broadcast(rb[:], rc[:])
                            nc.vector.tensor_mul(
                                attn_sb[t][64 * p:64 * p + 64, qsl],
                                pvs[0:DH, :], rb[:]
                            )
                    return tail

                prev_tail = None
                pending_outproj = []
                for qc in range(NQC):
                    # queue Q projections for this qc BEFORE the previous
                    # qc's outproj chunks: chain-start ensure() drains FIFO
                    # up to its Q, and outproj must stay behind the deferred
                    # normalize it reads from
                    for t in range(4):
                        if (t, qc) not in q_emitted:
                            q_emitted.add((t, qc))
                            for h in range(2):
                                fillers[("q", t, qc, h)] = (
                                    proj_half, ("q", wq_sb, xq_sb,
                                                qhT_sb[t], t, qc, h))
                    for key, args in pending_outproj:
                        fillers[key] = (outproj_half, args)
                    pending_outproj = []
                    for t in range(4):
                        # inputs this chain depends on must be emitted first
                        ensure_filler(("k", t, 3, 1))
                        ensure_filler(("q", t, qc, 1))
                        kh = khT_sb[t]
                        qh = qhT_sb[t]
                        qsl = slice(qc * W, (qc + 1) * W)
                        pv = None
                        ex_tiles = [None] * 16
                        for kb in range(16):
                            sc = scp.tile([128, 2 * W], F32, tag="sc",
                                          name=f"{pfx}sc_{t}_{qc}_{kb}")
                            ksl = slice(kb * 128, (kb + 1) * 128)
                            # two concurrent row-tiled matmuls (tiles T0/T8),
                            # each writing its own PSUM bank of sc
                            for p in range(2):
                                hsl = slice(64 * p, 64 * p + 64)
                                nc.tensor.matmul(
                                    sc[:, p * W:(p + 1) * W],
                                    kh[hsl, ksl],
                                    qh[hsl, qsl],
                                    start=True,
                                    stop=True,
                                )
                            ex = expp.tile([128, 2 * W], BF16, tag="exp",
                                           name=f"{pfx}ex_{t}_{qc}_{kb}")
                            ex_tiles[kb] = ex
                            nc.scalar.activation(
                                ex[:], sc[:], mybir.ActivationFunctionType.Exp,
                                scale=SCALE,
                            )
                            if kb == 0 and prev_tail is not None:
                                # previous chain's last pv + normalize land
                                # here, after this chain's first ACT is queued
                                prev_tail()
                                prev_tail = None
                            # pv accumulation for previous kb emitted after
                            # this kb's scores to keep ACT double-buffered
                            if kb > 0:
                                if pv is None:
                                    # allocated after the previous chain's
                                    # normalize is emitted so pool-slot
                                    # reuse dependencies are correct
                                    pv = [
                                        pvp.tile([DH + 1, W], F32, tag="pv",
                                                 name=f"{pfx}pv_{t}_{qc}_{p}")
                                        for p in range(2)
                                    ]
                                for p in range(2):
                                    nc.tensor.matmul(
                                        pv[p][:],
                                        vh_sb[kb - 1][:, 2 * t + p, :],
                                        ex_tiles[kb - 1][:, p * W:(p + 1) * W],
                                        start=(kb - 1 == 0),
                                        stop=False,
                                    )
                            if kb % filler_gap == filler_gap - 1:
                                emit_filler()
                        prev_tail = make_tail(t, qc, pv, ex_tiles[15])
                    # output projection for this qc's 4 row-blocks
                    if "c" in phases:
                        if qc < NQC - 1:
                            for qb in range(4 * qc, 4 * qc + 4):
                                for nt in range(2):
                                    for h in range(2):
                                        pending_outproj.append(
                                            (("y", qb, nt, h), (qb, nt, h)))
                        else:
                            # last chain's tail, leftover fillers, final
                            # outproj
                            prev_tail()
                            prev_tail = None
                            while emit_filler():
                                pass
                            for qb in range(4 * qc, 4 * qc + 4):
                                for nt in range(2):
                                    outproj_half(qb, nt, 0)
                                    outproj_half(qb, nt, 1)

    nc.finalize()
    return nc


def _get_nc():
    global _NC_CACHE
    if _NC_CACHE is None:
        _NC_CACHE = _build_nc()
    return _NC_CACHE


def kernel(q, k, v, wq, wk, wv, wo, _res_hook=None):
    q = np.asarray(q, dtype=np.float32)
    k = np.asarray(k, dtype=np.float32)
    v = np.asarray(v, dtype=np.float32)
    wq = np.asarray(wq, dtype=np.float32)
    wk = np.asarray(wk, dtype=np.float32)
    wv = np.asarray(wv, dtype=np.float32)
    wo = np.asarray(wo, dtype=np.float32)
    B = q.shape[0]

    nc = _get_nc()
    in_maps = []
    for c in range(N_CORES):
        b, g = c // 2, c % 2
        sl = slice(DL * g, DL * (g + 1))
        in_maps.append({
            "xq": np.ascontiguousarray(q[b].T).astype(ml_dtypes.bfloat16),
            "xk": np.ascontiguousarray(k[b].T).astype(ml_dtypes.bfloat16),
            "xv": np.ascontiguousarray(v[b].T).astype(ml_dtypes.bfloat16),
            "wqt": np.ascontiguousarray(wq[sl, :].T).astype(ml_dtypes.bfloat16),
            "wkt": np.ascontiguousarray(wk[sl, :].T).astype(ml_dtypes.bfloat16),
            "wvt": np.ascontiguousarray(wv[sl, :].T).astype(ml_dtypes.bfloat16),
            "wot": np.ascontiguousarray(wo[:, sl].T).astype(ml_dtypes.bfloat16),
        })

    res = run_bass_kernel_spmd(nc, in_maps, list(range(N_CORES)))
    if _res_hook is not None:
        _res_hook(res)

    out = np.empty((B, S, D), dtype=np.float32)
    for c in range(N_CORES):
        b, g = c // 2, c % 2
        yc = res.results[c]["y"]
        for ch in range(8):
            out[b, 256 * ch + 128 * g:256 * ch + 128 * (g + 1), :] = \
                yc[128 * ch:128 * (ch + 1), :]
    return out
